# revision 7
# baseline (speedup 1.0000x reference)
"""VQ codebook (cosine / normalized) kernel for Trainium2, 8 NeuronCores SPMD.

Reference computation (see problem):
  zf = transpose(z, bchw->bhwc).reshape(N, 64); zfn = l2norm(zf)
  wn = l2norm(weight)                    # [8192, 64]
  d = zfn @ wn.T; idx = argmax(d, 1)     # [N]
  z_q = wn[idx]  (straight-through => z_q_out == z_q numerically)
  loss = 1.01 * mean((z_q - zfn)**2)     # = 1.01 * mean_n(2 - 2*cos_n)/64
  encodings = one_hot(idx); perplexity from avg_probs

Device (per core, data-parallel over tokens, codebook replicated):
  - normalize codebook rows, build wnT [64, 8192] via PE transpose
  - scores = zf_shard @ wnT  (un-normalized zf: argmax-invariant)
  - argmax over 8192 per token (chunk maxes + max_index)
  - per-token loss term e = 2 - 2*smax/|zf|
Host: gather shards, build one-hot / z_q / scalars from device indices.
"""

import sys

sys.path.insert(0, "/opt/trn_rl_repo")

import numpy as np

import concourse.bass as bass
import concourse.mybir as mybir
from concourse.masks import make_identity
from concourse.tile import TileContext

F32 = mybir.dt.float32
I32 = mybir.dt.int32
U32 = mybir.dt.uint32

N_CORES = 8
B, C, H, W = 16, 64, 32, 32
N = B * H * W  # 16384 tokens
D = 64
K = 8192
TPC = N // N_CORES  # 2048 tokens per core
P = 128
NT = TPC // P  # 16 token tiles per core
NWT = K // P  # 64 codebook tiles
CHW = 512  # score chunk width (one PSUM bank)
NCH = K // CHW  # 16 chunks

_CACHE = {}

# Opcodes lowered as TPB_CTRL in walrus codegen: only 1 sync-wait slot.
_CTRL_OPCODES = {"Drain", "NoOp", "EventSemaphore", "AllEngineBarrier", "Halt"}


def split_waits(nc, max_compute=1):
    """Walrus in this env encodes a limited number of sync waits per
    instruction (1 for TPB_CTRL). Split excess waits onto preceding
    InstNoOp instructions on the same engine (engine waits execute in
    order, so this is semantics-preserving)."""
    n_new = 0
    for f in nc.m.functions:
        for bb in f.blocks:
            insts = bb.instructions
            i = 0
            while i < len(insts):
                inst = insts[i]
                si = inst.sync_info
                waits = list(si.on_wait) if (si and si.on_wait) else []
                cap = 1 if str(inst.opcode) in _CTRL_OPCODES else max_compute
                if len(waits) > cap:
                    keep = waits[-cap:]
                    extra = waits[:-cap]
                    pos = i
                    for j in range(0, len(extra)):
                        nop = mybir.InstNoOp(name=f"{inst.name}-wnop{j}")
                        nop.engine = inst.engine
                        nop.sync_info = mybir.SyncInfo(
                            on_wait=[extra[j]], on_update=[]
                        )
                        insts.insert(pos, nop)
                        pos += 1
                        n_new += 1
                    si.on_wait = keep
                    i = pos
                i += 1
    return n_new


def build_nc(split=True):
    nc = bass.Bass()

    zt_d = nc.declare_dram_parameter("zt", [D, TPC], F32, isOutput=False)
    ztok_d = nc.declare_dram_parameter("ztok", [TPC, D], F32, isOutput=False)
    w_d = nc.declare_dram_parameter("weight", [K, D], F32, isOutput=False)
    idx_d = nc.declare_dram_parameter("idx_out", [P, NT], I32, isOutput=True)
    loss_d = nc.declare_dram_parameter("loss_e", [P, NT], F32, isOutput=True)
    wn_d = nc.declare_dram_parameter("wn_out", [K, D], F32, isOutput=True)

    mult = mybir.AluOpType.mult
    add = mybir.AluOpType.add

    with TileContext(nc) as tc:
        with (
            tc.tile_pool(name="const", bufs=1) as const,
            tc.tile_pool(name="big", bufs=1) as big,
            tc.tile_pool(name="w", bufs=3) as wpool,
            tc.tile_pool(name="scores", bufs=2) as spool,
            tc.tile_pool(name="small", bufs=4) as small,
            tc.tile_pool(name="z", bufs=3) as zpool,
            tc.tile_pool(name="mm", bufs=6, space="PSUM") as psum,
            tc.tile_pool(name="pt", bufs=2, space="PSUM") as psum_t,
        ):
            ident = const.tile([P, P], F32)
            make_identity(nc, ident[:])

            wnT = big.tile([D, K], F32, tag="wnT")
            zt_sb = big.tile([D, TPC], F32, tag="zt")
            idx_all = big.tile([P, NT], I32, tag="idx")
            loss_all = big.tile([P, NT], F32, tag="loss")

            nc.sync.dma_start(zt_sb[:], zt_d[:, :])

            # --- codebook normalize + transpose ---
            for wt in range(NWT):
                w_t = wpool.tile([P, D], F32, tag="w_t")
                nc.sync.dma_start(w_t[:], w_d[wt * P : (wt + 1) * P, :])
                sq = wpool.tile([P, D], F32, tag="sq")
                n2 = small.tile([P, 1], F32, tag="n2")
                nc.vector.tensor_mul(sq[:], w_t[:], w_t[:])
                nc.vector.reduce_sum(n2[:], sq[:], axis=mybir.AxisListType.X)
                nrm = small.tile([P, 1], F32, tag="nrm")
                nc.scalar.sqrt(nrm[:], n2[:])
                rin = small.tile([P, 1], F32, tag="rin")
                nc.vector.reciprocal(rin[:], nrm[:])
                wn_t = wpool.tile([P, D], F32, tag="wn_t")
                nc.vector.tensor_scalar_mul(wn_t[:], w_t[:], rin[:])
                nc.sync.dma_start(wn_d[wt * P : (wt + 1) * P, :], wn_t[:])
                pt = psum_t.tile([D, P], F32, tag="pt")
                nc.tensor.transpose(pt[:], wn_t[:], ident[:])
                nc.scalar.copy(wnT[:, wt * P : (wt + 1) * P], pt[:])

            # --- per token tile: scores, argmax, loss term ---
            for t in range(NT):
                scores = spool.tile([P, K], F32, tag="scores")
                for c in range(NCH):
                    ps = psum.tile([P, CHW], F32, tag="mm")
                    nc.tensor.matmul(
                        ps[:],
                        lhsT=zt_sb[:, t * P : (t + 1) * P],
                        rhs=wnT[:, c * CHW : (c + 1) * CHW],
                        start=True,
                        stop=True,
                    )
                    nc.scalar.copy(scores[:, c * CHW : (c + 1) * CHW], ps[:])

                cm = small.tile([P, NCH], F32, tag="cm")
                nc.vector.reduce_max(
                    cm[:],
                    scores[:].rearrange("p (c w) -> p c w", w=CHW),
                    axis=mybir.AxisListType.X,
                )
                gmax = small.tile([P, 1], F32, tag="gmax")
                nc.vector.reduce_max(gmax[:], cm[:], axis=mybir.AxisListType.X)

                in8 = small.tile([P, 8], F32, tag="in8")
                nc.vector.memset(in8[:], -3.0e38)
                nc.vector.tensor_copy(in8[:, 0:1], gmax[:])
                idx8 = small.tile([P, 8], U32, tag="idx8")
                nc.vector.max_index(idx8[:], in8[:], scores[:])
                nc.vector.tensor_copy(idx_all[:, t : t + 1], idx8[:, 0:1])

                ztk = zpool.tile([P, D], F32, tag="ztk")
                nc.sync.dma_start(ztk[:], ztok_d[t * P : (t + 1) * P, :])
                zsq = zpool.tile([P, D], F32, tag="zsq")
                zn2 = small.tile([P, 1], F32, tag="zn2")
                nc.vector.tensor_mul(zsq[:], ztk[:], ztk[:])
                nc.vector.reduce_sum(zn2[:], zsq[:], axis=mybir.AxisListType.X)
                znr = small.tile([P, 1], F32, tag="znr")
                nc.scalar.sqrt(znr[:], zn2[:])
                zri = small.tile([P, 1], F32, tag="zri")
                nc.vector.reciprocal(zri[:], znr[:])
                cosv = small.tile([P, 1], F32, tag="cos")
                nc.vector.tensor_mul(cosv[:], gmax[:], zri[:])
                nc.vector.tensor_scalar(
                    out=loss_all[:, t : t + 1],
                    in0=cosv[:],
                    scalar1=-2.0,
                    scalar2=2.0,
                    op0=mult,
                    op1=add,
                )

            nc.sync.dma_start(idx_d[:, :], idx_all[:])
            nc.sync.dma_start(loss_d[:, :], loss_all[:])

    if split:
        split_waits(nc)
    return nc


def make_in_maps(z, weight):
    z = np.ascontiguousarray(z, dtype=np.float32)
    w = np.ascontiguousarray(weight, dtype=np.float32)
    zf = np.ascontiguousarray(z.transpose(0, 2, 3, 1).reshape(N, D))
    in_maps = []
    for c in range(N_CORES):
        sh = zf[c * TPC : (c + 1) * TPC]
        in_maps.append(
            {
                "zt": np.ascontiguousarray(sh.T),
                "ztok": np.ascontiguousarray(sh),
                "weight": w,
            }
        )
    return in_maps


def assemble(results):
    idx_parts = []
    loss_sum = 0.0
    for r in results:
        idx_parts.append(np.ascontiguousarray(r["idx_out"].T).reshape(-1))
        loss_sum += r["loss_e"].astype(np.float64).sum()
    idx = np.concatenate(idx_parts).astype(np.int32)
    wn = np.asarray(results[0]["wn_out"], dtype=np.float32)

    z_q_flat = wn[idx]
    z_q_out = np.ascontiguousarray(
        z_q_flat.reshape(B, H, W, C).transpose(0, 3, 1, 2)
    ).astype(np.float32)

    encodings = np.zeros((N, K), dtype=np.float32)
    encodings[np.arange(N), idx] = 1.0

    loss = np.float32(1.01 * loss_sum / (N * D))

    counts = np.bincount(idx, minlength=K).astype(np.float64)
    avg_probs = counts / N
    perplexity = np.float32(np.exp(-np.sum(avg_probs * np.log(avg_probs + 1e-10))))

    return z_q_out, idx, loss, encodings, perplexity


def kernel(z, weight):
    from concourse.bass_utils import run_bass_kernel_spmd

    if "nc" not in _CACHE:
        _CACHE["nc"] = build_nc()
    nc = _CACHE["nc"]
    in_maps = make_in_maps(z, weight)
    res = run_bass_kernel_spmd(nc, in_maps, list(range(N_CORES)))
    return assemble(res.results)
